# revision 29
# baseline (speedup 1.0000x reference)
"""Chebyshev graph-conv (gnn_message_passing) Trainium2 kernel.

Reference computation (see problem):
    x0 = inputs [1,8,V,8,8,8] -> [V, Fin*B*X*Y*Z]
    Chebyshev recurrence with sparse Laplacian (COO, 8 entries/row), K=5
    out = einsum('kvfbxyz,kfo->bovxyz', cheb, weight) + bias

Sharding: dense dim D = Fin*XYZ split over the XYZ axis across 8 cores
(64 spatial positions per core -> local D = 64*8 = 512, laid out d = s*8+f).

Per-core algorithm (all compute on device):
  - spmv via SWDGE dma_gather of x rows from HBM (indices are runtime data)
    followed by PE selection-matmuls that fold the vals in and do the
    8-way segment sum into PSUM.
  - Chebyshev combine (x_next = 2*psum - x_prev) on DVE.
  - cheb terms transposed with PE transpose-mode; output einsum over (k,f)
    runs as PE matmuls with block-diagonal weight-selection matrices,
    accumulated into SBUF (bias folded into the k=0 pass).
"""

import sys

for _p in ("/opt/trn_rl_repo", "/root/.axon_site/_ro/trn_rl_repo"):
    if _p not in sys.path:
        sys.path.append(_p)

import numpy as np

V = 2562
DEG = 8
B, FIN, FOUT, K = 1, 8, 16, 5
XYZ = 512
NCORES = 8
SLOC = XYZ // NCORES  # 64 spatial positions per core
D = SLOC * FIN  # 512 local dense dim, d = s_loc*8 + f

VP = 2688  # V padded to 21*128
NT = VP // 128  # 21 v-tiles
EPAD = VP * DEG  # 21504 padded edges
NCHUNK = EPAD // 128  # 168 edge chunks of 128 (16 v's each)
NVCH = 6  # v-chunks of up to 4 v-tiles (5*4 + 1)

_COMPILED = [None]
LAST_RESULT = [None]


def _build(gch=8):
    import os as _os

    _skip_out = bool(int(_os.environ.get("SKIP_OUT", "0")))
    _skip_t = bool(int(_os.environ.get("SKIP_T", "0")))
    from contextlib import ExitStack

    import concourse.mybir as mybir
    import concourse.tile as tile
    from concourse import bacc

    fp32 = mybir.dt.float32
    i16 = mybir.dt.int16
    Alu = mybir.AluOpType

    nc = bacc.Bacc(None, target_bir_lowering=False)

    bf16 = mybir.dt.bfloat16
    f32r = mybir.dt.float32r
    x0g = nc.dram_tensor("x0g", [VP, D], bf16, kind="ExternalInput")
    idxd = nc.dram_tensor("idx", [128, NT * gch * 8], i16, kind="ExternalInput")
    seld = nc.dram_tensor("sel", [128, NT * gch, 128], bf16, kind="ExternalInput")
    wseld = nc.dram_tensor("wsel", [128, K * 2, 128], fp32, kind="ExternalInput")
    biasd = nc.dram_tensor("biasx", [128, 2], fp32, kind="ExternalInput")
    outd = nc.dram_tensor("outT", [8, 128, VP], fp32, kind="ExternalOutput")

    from concourse.masks import make_identity

    with ExitStack() as ctx:
        tc = ctx.enter_context(tile.TileContext(nc))
        const = ctx.enter_context(tc.tile_pool(name="const", bufs=1))
        accp = ctx.enter_context(tc.tile_pool(name="acc", bufs=1))
        dram = ctx.enter_context(tc.tile_pool(name="dram", bufs=1, space="DRAM"))
        gp = ctx.enter_context(tc.tile_pool(name="g", bufs=4))
        sp = ctx.enter_context(tc.tile_pool(name="stream", bufs=4))
        xtp = ctx.enter_context(tc.tile_pool(name="xt", bufs=3))
        pp = ctx.enter_context(tc.tile_pool(name="psum", bufs=2, space="PSUM"))
        ppv = ctx.enter_context(tc.tile_pool(name="psumv", bufs=3, space="PSUM"))

        SEL = const.tile([128, NT * gch, 128], bf16)
        WSEL = const.tile([128, K * 2, 128], f32r)
        IDX = const.tile([128, NT * gch * 8], i16)
        BIA = const.tile([128, 2], fp32)
        IDN = const.tile([128, 128], fp32)
        IDNB = const.tile([128, 128], bf16)
        nq = NT * gch
        for q in range(4):
            qs = slice(q * nq // 4, (q + 1) * nq // 4)
            nc.sync.dma_start(SEL[:, qs, :], seld[:, qs, :])
        WSELF = sp.tile([128, K * 2 * 128], fp32, tag="wself", name="WSELF", bufs=1)
        nc.sync.dma_start(WSELF[:], wseld[:].rearrange("p a b -> p (a b)"))
        nc.vector.tensor_copy(
            WSEL[:].rearrange("p a b -> p (a b)"), WSELF[:]
        )
        nc.sync.dma_start(IDX[:], idxd[:])
        nc.sync.dma_start(BIA[:], biasd[:])
        make_identity(nc, IDN[:])
        nc.vector.tensor_copy(IDNB[:], IDN[:])

        ACC = [accp.tile([128, VP], fp32, tag=f"acc{i}", name=f"acc{i}") for i in range(8)]
        if _skip_out or _skip_t:
            for i in range(8):
                nc.vector.memset(ACC[i][:], 0.0)
        xgb = [dram.tile([VP, D], bf16, tag=f"xgb{i}", name=f"xgb{i}") for i in range(3)]

        def transpose_to(xv, xTt, vl, dt=fp32):
            """xv [128 v, 512 d] -> xTt[:, t, 128*vl:+128] for t in 0..3."""
            if _skip_t:
                return
            psT = pp.tile([128, 512], dt, tag="psT")
            for t in range(4):
                s = slice(128 * t, 128 * (t + 1))
                nc.tensor.transpose(
                    psT[:, s], xv[:, s], IDN[:] if dt == fp32 else IDNB[:]
                )
            for t in range(4):
                nc.scalar.copy(
                    xTt[:, t, 128 * vl : 128 * (vl + 1)],
                    psT[:, 128 * t : 128 * (t + 1)],
                )

        def out_stage(k, ch, nvt, xTt):
            """ACC[(t,h)][:, chunk] (+)= WSEL_k,h^T @ xT_t  (+bias at k=0)."""
            if _skip_out or _skip_t:
                return
            n = nvt * 128
            cs = slice(512 * ch, 512 * ch + n)
            for t in range(4):
                for h in range(2):
                    i = t * 2 + h
                    psO = pp.tile([128, 512], fp32, tag="psO")
                    nc.tensor.matmul(
                        psO[:, :n],
                        WSEL[:, k * 2 + h, :],
                        xTt[:, t, :n],
                        start=True,
                        stop=True,
                    )
                    if k == 0:
                        nc.vector.tensor_scalar(
                            ACC[i][:, cs], psO[:, :n], BIA[:, h : h + 1], None, Alu.add
                        )
                    else:
                        nc.vector.tensor_tensor(
                            ACC[i][:, cs], ACC[i][:, cs], psO[:, :n], Alu.add
                        )
                    if k == K - 1:
                        nc.sync.dma_start(outd[i, :, cs], ACC[i][:, cs])

        # ---- k = 0: cheb_0 = x0 ----
        for ch in range(NVCH):
            nvt = 4 if ch < 5 else 1
            xTt = xtp.tile([128, 4, 512], f32r, tag="xTt")
            for vl in range(nvt):
                vt = 4 * ch + vl
                xv0 = sp.tile([128, D], bf16, tag="xv0", bufs=2)
                nc.sync.dma_start(xv0[:], x0g[128 * vt : 128 * (vt + 1), :])
                transpose_to(xv0, xTt, vl, dt=bf16)
            out_stage(0, ch, nvt, xTt)

        # ---- k = 1..4: x_k = 2 L x_{k-1} - x_{k-2}   (k=1: x_1 = L x_0) ----
        for k in range(1, K):
            src = x0g if k == 1 else xgb[(k - 2) % 3]
            prev = None if k == 1 else (x0g if k == 2 else xgb[(k - 3) % 3])
            dstb = xgb[(k - 1) % 3]
            for ch in range(NVCH):
                nvt = 4 if ch < 5 else 1
                xTt = xtp.tile([128, 4, 512], f32r, tag="xTt")
                for vl in range(nvt):
                    vt = 4 * ch + vl
                    psV = ppv.tile([128, 512], fp32, tag="psV")
                    g = gp.tile([128, gch, D], bf16, tag="g")
                    with tc.high_priority(offset=400):
                        nc.gpsimd.dma_gather(
                            g[:],
                            src[:],
                            IDX[:, gch * 8 * vt : gch * 8 * (vt + 1)],
                            num_idxs=gch * 128,
                            num_idxs_reg=gch * 128,
                            elem_size=D,
                        )
                    for j in range(gch):
                        nc.tensor.matmul(
                            psV[:],
                            SEL[:, gch * vt + j, :],
                            g[:, j, :],
                            start=(j == 0),
                            stop=(j == gch - 1),
                        )
                    xv = sp.tile([128, D], fp32, tag="xv")
                    with tc.high_priority(offset=600):
                        if k == 1:
                            nc.vector.tensor_copy(xv[:], psV[:])
                        else:
                            pv = sp.tile([128, D], bf16, tag="pv")
                            nc.sync.dma_start(
                                pv[:], prev[128 * vt : 128 * (vt + 1), :]
                            )
                            nc.vector.scalar_tensor_tensor(
                                xv[:], psV[:], 2.0, pv[:], Alu.mult, Alu.subtract
                            )
                        if k < K - 1:
                            xvb = sp.tile([128, D], bf16, tag="xvb")
                            nc.vector.tensor_copy(xvb[:], xv[:])
                            nc.sync.dma_start(
                                dstb[128 * vt : 128 * (vt + 1), :], xvb[:]
                            )
                    transpose_to(xv, xTt, vl)
                out_stage(k, ch, nvt, xTt)


    nc.compile()
    return nc


def _host_prep(inputs, lap_rows, lap_cols, lap_vals, weight, bias):
    inputs = np.asarray(inputs, dtype=np.float32)
    lap_rows = np.asarray(lap_rows)
    lap_cols = np.asarray(lap_cols)
    lap_vals = np.asarray(lap_vals, dtype=np.float32)
    weight = np.asarray(weight, dtype=np.float32)
    bias = np.asarray(bias, dtype=np.float32)

    nnz = lap_rows.shape[0]
    order = np.argsort(lap_rows, kind="stable")
    srows = lap_rows[order]
    assert np.array_equal(
        np.repeat(np.arange(V, dtype=srows.dtype), DEG), srows
    ), "expected exactly DEG entries per row"
    e_cols = np.zeros(EPAD, np.int64)
    e_vals = np.zeros(EPAD, np.float32)
    e_cols[:nnz] = lap_cols[order]
    e_vals[:nnz] = lap_vals[order]

    # per-v-tile dedup: gather each unique col once; SEL folds vals and
    # scatters every (unique col -> output v) pair of the tile
    uniq = []
    for vt in range(NT):
        ecols = e_cols[1024 * vt : 1024 * (vt + 1)]
        uniq.append(np.unique(ecols))
    gch = max(2, max((len(u) + 127) // 128 for u in uniq))

    idx_np = np.zeros((128, NT * gch * 8), np.int16)
    sel_np = np.zeros((128, NT * gch, 128), np.float32)
    for vt in range(NT):
        u = uniq[vt]
        slots = np.zeros(gch * 128, np.int64)
        slots[: len(u)] = u
        w = slots.reshape(gch * 8, 16).T.astype(np.int16)  # wrapped-16
        idx_np[:, gch * 8 * vt : gch * 8 * (vt + 1)] = np.tile(w, (8, 1))
        col2slot = np.zeros(V + 1, np.int64)
        col2slot[u] = np.arange(len(u))
        ecols = e_cols[1024 * vt : 1024 * (vt + 1)]
        evals = e_vals[1024 * vt : 1024 * (vt + 1)]
        sl = col2slot[ecols]
        m = np.arange(1024) // DEG  # output row within v-tile
        np.add.at(sel_np, (sl % 128, gch * vt + sl // 128, m), evals)
    import ml_dtypes

    sel_np = sel_np.astype(ml_dtypes.bfloat16)

    # output-stage weight selection: rows p=s_loc*8+f, cols q=s_loc*8+o
    wsel_np = np.zeros((128, K * 2, 128), np.float32)
    sl = np.arange(16)
    for k in range(K):
        for h in range(2):
            for f in range(FIN):
                for o in range(8):
                    wsel_np[sl * 8 + f, k * 2 + h, sl * 8 + o] = weight[k, f, 8 * h + o]

    bias_np = np.zeros((128, 2), np.float32)
    p = np.arange(128)
    for h in range(2):
        bias_np[p, h] = bias[8 * h + p % 8]

    # x0 shards: [V, s, f] per core
    xt = inputs.reshape(FIN, V, XYZ).transpose(1, 2, 0)  # [V, 512, 8]
    x0s = []
    for m in range(NCORES):
        x0m = np.zeros((VP, D), np.float32)
        x0m[:V] = xt[:, SLOC * m : SLOC * (m + 1), :].reshape(V, D)
        x0s.append(x0m)
    return x0s, idx_np, sel_np, wsel_np, bias_np


def kernel(inputs, lap_rows, lap_cols, lap_vals, weight, bias):
    import ml_dtypes as _ml

    from concourse.bass_utils import run_bass_kernel_spmd

    x0s, idx_np, sel_np, wsel_np, bias_np = _host_prep(
        inputs, lap_rows, lap_cols, lap_vals, weight, bias
    )

    gch = idx_np.shape[1] // (NT * 8)
    if _COMPILED[0] is None or _COMPILED[0][0] != gch:
        _COMPILED[0] = (gch, _build(gch))
    nc = _COMPILED[0][1]

    in_maps = [
        {
            "x0g": x0s[m].astype(_ml.bfloat16),
            "idx": idx_np,
            "sel": sel_np,
            "wsel": wsel_np,
            "biasx": bias_np,
        }
        for m in range(NCORES)
    ]
    import os

    trace = bool(int(os.environ.get("KERNEL_TRACE", "0")))
    res = run_bass_kernel_spmd(
        nc, in_maps, core_ids=list(range(NCORES)), trace=trace
    )
    LAST_RESULT[0] = res

    # unshard: outT [8=(t,h), 128=(s_loc,o_loc), VP] per core
    parts = []
    for m in range(NCORES):
        r = res.results[m]["outT"]  # [8, 128, VP]
        r = r.reshape(4, 2, 16, 8, VP)[:, :, :, :, :V]  # [t, h, sl, ol, v]
        # o = 8h + ol ; s_local_in_core = 16t + sl
        r = r.transpose(1, 3, 4, 0, 2).reshape(FOUT, V, SLOC)  # [o, v, s]
        parts.append(r)
    out = np.concatenate(parts, axis=2)  # [o, v, 512]
    return np.ascontiguousarray(
        out.reshape(1, FOUT, V, 8, 8, 8).astype(np.float32)
    )


# revision 30
# speedup vs baseline: 1.0050x; 1.0050x over previous
"""Chebyshev graph-conv (gnn_message_passing) Trainium2 kernel.

Reference computation (see problem):
    x0 = inputs [1,8,V,8,8,8] -> [V, Fin*B*X*Y*Z]
    Chebyshev recurrence with sparse Laplacian (COO, 8 entries/row), K=5
    out = einsum('kvfbxyz,kfo->bovxyz', cheb, weight) + bias

Sharding: dense dim D = Fin*XYZ split over the XYZ axis across 8 cores
(64 spatial positions per core -> local D = 64*8 = 512, laid out d = s*8+f).

Per-core algorithm (all compute on device):
  - spmv via SWDGE dma_gather of x rows from HBM (indices are runtime data)
    followed by PE selection-matmuls that fold the vals in and do the
    8-way segment sum into PSUM.
  - Chebyshev combine (x_next = 2*psum - x_prev) on DVE.
  - cheb terms transposed with PE transpose-mode; output einsum over (k,f)
    runs as PE matmuls with block-diagonal weight-selection matrices,
    accumulated into SBUF (bias folded into the k=0 pass).
"""

import sys

for _p in ("/opt/trn_rl_repo", "/root/.axon_site/_ro/trn_rl_repo"):
    if _p not in sys.path:
        sys.path.append(_p)

import numpy as np

V = 2562
DEG = 8
B, FIN, FOUT, K = 1, 8, 16, 5
XYZ = 512
NCORES = 8
SLOC = XYZ // NCORES  # 64 spatial positions per core
D = SLOC * FIN  # 512 local dense dim, d = s_loc*8 + f

VP = 2688  # V padded to 21*128
NT = VP // 128  # 21 v-tiles
EPAD = VP * DEG  # 21504 padded edges
NCHUNK = EPAD // 128  # 168 edge chunks of 128 (16 v's each)
NVCH = 6  # v-chunks of up to 4 v-tiles (5*4 + 1)

_COMPILED = [None]
LAST_RESULT = [None]


def _build(gch=8):
    import os as _os

    _skip_out = bool(int(_os.environ.get("SKIP_OUT", "0")))
    _skip_t = bool(int(_os.environ.get("SKIP_T", "0")))
    from contextlib import ExitStack

    import concourse.mybir as mybir
    import concourse.tile as tile
    from concourse import bacc

    fp32 = mybir.dt.float32
    i16 = mybir.dt.int16
    Alu = mybir.AluOpType

    nc = bacc.Bacc(None, target_bir_lowering=False)

    bf16 = mybir.dt.bfloat16
    f32r = mybir.dt.float32r
    x0g = nc.dram_tensor("x0g", [VP, D], bf16, kind="ExternalInput")
    idxd = nc.dram_tensor("idx", [128, NT * gch * 8], i16, kind="ExternalInput")
    seld = nc.dram_tensor("sel", [128, NT * gch, 128], bf16, kind="ExternalInput")
    wseld = nc.dram_tensor("wsel", [128, K * 2, 128], fp32, kind="ExternalInput")
    biasd = nc.dram_tensor("biasx", [128, 2], fp32, kind="ExternalInput")
    outd = nc.dram_tensor("outT", [8, 128, VP], fp32, kind="ExternalOutput")

    from concourse.masks import make_identity

    with ExitStack() as ctx:
        tc = ctx.enter_context(tile.TileContext(nc))
        const = ctx.enter_context(tc.tile_pool(name="const", bufs=1))
        accp = ctx.enter_context(tc.tile_pool(name="acc", bufs=1))
        dram = ctx.enter_context(tc.tile_pool(name="dram", bufs=1, space="DRAM"))
        gp = ctx.enter_context(tc.tile_pool(name="g", bufs=4))
        sp = ctx.enter_context(tc.tile_pool(name="stream", bufs=4))
        xtp = ctx.enter_context(tc.tile_pool(name="xt", bufs=3))
        pp = ctx.enter_context(tc.tile_pool(name="psum", bufs=2, space="PSUM"))
        ppv = ctx.enter_context(tc.tile_pool(name="psumv", bufs=3, space="PSUM"))

        SEL = const.tile([128, NT * gch, 128], bf16)
        WSEL = const.tile([128, K * 2, 128], f32r)
        IDX = const.tile([128, NT * gch * 8], i16)
        BIA = const.tile([128, 2], fp32)
        IDN = const.tile([128, 128], fp32)
        IDNB = const.tile([128, 128], bf16)
        nq = NT * gch
        for q in range(4):
            qs = slice(q * nq // 4, (q + 1) * nq // 4)
            nc.sync.dma_start(SEL[:, qs, :], seld[:, qs, :])
        WSELF = sp.tile([128, K * 2 * 128], fp32, tag="wself", name="WSELF", bufs=1)
        nc.sync.dma_start(WSELF[:], wseld[:].rearrange("p a b -> p (a b)"))
        nc.vector.tensor_copy(
            WSEL[:].rearrange("p a b -> p (a b)"), WSELF[:]
        )
        nc.sync.dma_start(IDX[:], idxd[:])
        nc.sync.dma_start(BIA[:], biasd[:])
        make_identity(nc, IDN[:])
        nc.vector.tensor_copy(IDNB[:], IDN[:])

        ACC = [accp.tile([128, VP], fp32, tag=f"acc{i}", name=f"acc{i}") for i in range(8)]
        if _skip_out or _skip_t:
            for i in range(8):
                nc.vector.memset(ACC[i][:], 0.0)
        xgb = [dram.tile([VP, D], bf16, tag=f"xgb{i}", name=f"xgb{i}") for i in range(3)]

        def transpose_to(xv, xTt, vl, dt=fp32):
            """xv [128 v, 512 d] -> xTt[:, t, 128*vl:+128] for t in 0..3."""
            if _skip_t:
                return
            psT = pp.tile([128, 512], dt, tag="psT")
            for t in range(4):
                s = slice(128 * t, 128 * (t + 1))
                nc.tensor.transpose(
                    psT[:, s], xv[:, s], IDN[:] if dt == fp32 else IDNB[:]
                )
            for t in range(4):
                nc.scalar.copy(
                    xTt[:, t, 128 * vl : 128 * (vl + 1)],
                    psT[:, 128 * t : 128 * (t + 1)],
                )

        def out_stage(k, ch, nvt, xTt):
            """ACC[(t,h)][:, chunk] (+)= WSEL_k,h^T @ xT_t  (+bias at k=0)."""
            if _skip_out or _skip_t:
                return
            n = nvt * 128
            cs = slice(512 * ch, 512 * ch + n)
            for t in range(4):
                for h in range(2):
                    i = t * 2 + h
                    psO = pp.tile([128, 512], fp32, tag="psO")
                    nc.tensor.matmul(
                        psO[:, :n],
                        WSEL[:, k * 2 + h, :],
                        xTt[:, t, :n],
                        start=True,
                        stop=True,
                    )
                    if k == 0:
                        nc.vector.tensor_scalar(
                            ACC[i][:, cs], psO[:, :n], BIA[:, h : h + 1], None, Alu.add
                        )
                    else:
                        nc.vector.tensor_tensor(
                            ACC[i][:, cs], ACC[i][:, cs], psO[:, :n], Alu.add
                        )
                    if k == K - 1:
                        nc.sync.dma_start(outd[i, :, cs], ACC[i][:, cs])

        # ---- k = 0: cheb_0 = x0 ----
        for ch in range(NVCH):
            nvt = 4 if ch < 5 else 1
            xTt = xtp.tile([128, 4, 512], f32r, tag="xTt")
            for vl in range(nvt):
                vt = 4 * ch + vl
                xv0 = sp.tile([128, D], bf16, tag="xv0", bufs=2)
                nc.sync.dma_start(xv0[:], x0g[128 * vt : 128 * (vt + 1), :])
                transpose_to(xv0, xTt, vl, dt=bf16)
            out_stage(0, ch, nvt, xTt)

        # ---- k = 1..4: x_k = 2 L x_{k-1} - x_{k-2}   (k=1: x_1 = L x_0) ----
        for k in range(1, K):
            src = x0g if k == 1 else xgb[(k - 2) % 3]
            prev = None if k == 1 else (x0g if k == 2 else xgb[(k - 3) % 3])
            dstb = xgb[(k - 1) % 3]
            for ch in range(NVCH):
                nvt = 4 if ch < 5 else 1
                xTt = xtp.tile([128, 4, 512], f32r, tag="xTt")
                for vl in range(nvt):
                    vt = 4 * ch + vl
                    psV = ppv.tile([128, 512], fp32, tag="psV")
                    g = gp.tile([128, gch, D], bf16, tag="g")
                    with tc.high_priority(offset=400):
                        nc.gpsimd.dma_gather(
                            g[:],
                            src[:],
                            IDX[:, gch * 8 * vt : gch * 8 * (vt + 1)],
                            num_idxs=gch * 128,
                            num_idxs_reg=gch * 128,
                            elem_size=D,
                        )
                    for j in range(gch):
                        nc.tensor.matmul(
                            psV[:],
                            SEL[:, gch * vt + j, :],
                            g[:, j, :],
                            start=(j == 0),
                            stop=(j == gch - 1),
                        )
                    xv = sp.tile([128, D], fp32, tag="xv")
                    with tc.high_priority(offset=600):
                        if k == 1:
                            nc.vector.tensor_copy(xv[:], psV[:])
                        else:
                            pv = sp.tile([128, D], bf16, tag="pv", bufs=6)
                            with tc.high_priority(offset=1200):
                                nc.sync.dma_start(
                                    pv[:], prev[128 * vt : 128 * (vt + 1), :]
                                )
                            nc.vector.scalar_tensor_tensor(
                                xv[:], psV[:], 2.0, pv[:], Alu.mult, Alu.subtract
                            )
                        if k < K - 1:
                            xvb = sp.tile([128, D], bf16, tag="xvb")
                            nc.vector.tensor_copy(xvb[:], xv[:])
                            nc.sync.dma_start(
                                dstb[128 * vt : 128 * (vt + 1), :], xvb[:]
                            )
                    transpose_to(xv, xTt, vl)
                out_stage(k, ch, nvt, xTt)


    nc.compile()
    return nc


def _host_prep(inputs, lap_rows, lap_cols, lap_vals, weight, bias):
    inputs = np.asarray(inputs, dtype=np.float32)
    lap_rows = np.asarray(lap_rows)
    lap_cols = np.asarray(lap_cols)
    lap_vals = np.asarray(lap_vals, dtype=np.float32)
    weight = np.asarray(weight, dtype=np.float32)
    bias = np.asarray(bias, dtype=np.float32)

    nnz = lap_rows.shape[0]
    order = np.argsort(lap_rows, kind="stable")
    srows = lap_rows[order]
    assert np.array_equal(
        np.repeat(np.arange(V, dtype=srows.dtype), DEG), srows
    ), "expected exactly DEG entries per row"
    e_cols = np.zeros(EPAD, np.int64)
    e_vals = np.zeros(EPAD, np.float32)
    e_cols[:nnz] = lap_cols[order]
    e_vals[:nnz] = lap_vals[order]

    # per-v-tile dedup: gather each unique col once; SEL folds vals and
    # scatters every (unique col -> output v) pair of the tile
    uniq = []
    for vt in range(NT):
        ecols = e_cols[1024 * vt : 1024 * (vt + 1)]
        uniq.append(np.unique(ecols))
    gch = max(2, max((len(u) + 127) // 128 for u in uniq))

    idx_np = np.zeros((128, NT * gch * 8), np.int16)
    sel_np = np.zeros((128, NT * gch, 128), np.float32)
    for vt in range(NT):
        u = uniq[vt]
        slots = np.zeros(gch * 128, np.int64)
        slots[: len(u)] = u
        w = slots.reshape(gch * 8, 16).T.astype(np.int16)  # wrapped-16
        idx_np[:, gch * 8 * vt : gch * 8 * (vt + 1)] = np.tile(w, (8, 1))
        col2slot = np.zeros(V + 1, np.int64)
        col2slot[u] = np.arange(len(u))
        ecols = e_cols[1024 * vt : 1024 * (vt + 1)]
        evals = e_vals[1024 * vt : 1024 * (vt + 1)]
        sl = col2slot[ecols]
        m = np.arange(1024) // DEG  # output row within v-tile
        np.add.at(sel_np, (sl % 128, gch * vt + sl // 128, m), evals)
    import ml_dtypes

    sel_np = sel_np.astype(ml_dtypes.bfloat16)

    # output-stage weight selection: rows p=s_loc*8+f, cols q=s_loc*8+o
    wsel_np = np.zeros((128, K * 2, 128), np.float32)
    sl = np.arange(16)
    for k in range(K):
        for h in range(2):
            for f in range(FIN):
                for o in range(8):
                    wsel_np[sl * 8 + f, k * 2 + h, sl * 8 + o] = weight[k, f, 8 * h + o]

    bias_np = np.zeros((128, 2), np.float32)
    p = np.arange(128)
    for h in range(2):
        bias_np[p, h] = bias[8 * h + p % 8]

    # x0 shards: [V, s, f] per core
    xt = inputs.reshape(FIN, V, XYZ).transpose(1, 2, 0)  # [V, 512, 8]
    x0s = []
    for m in range(NCORES):
        x0m = np.zeros((VP, D), np.float32)
        x0m[:V] = xt[:, SLOC * m : SLOC * (m + 1), :].reshape(V, D)
        x0s.append(x0m)
    return x0s, idx_np, sel_np, wsel_np, bias_np


def kernel(inputs, lap_rows, lap_cols, lap_vals, weight, bias):
    import ml_dtypes as _ml

    from concourse.bass_utils import run_bass_kernel_spmd

    x0s, idx_np, sel_np, wsel_np, bias_np = _host_prep(
        inputs, lap_rows, lap_cols, lap_vals, weight, bias
    )

    gch = idx_np.shape[1] // (NT * 8)
    if _COMPILED[0] is None or _COMPILED[0][0] != gch:
        _COMPILED[0] = (gch, _build(gch))
    nc = _COMPILED[0][1]

    in_maps = [
        {
            "x0g": x0s[m].astype(_ml.bfloat16),
            "idx": idx_np,
            "sel": sel_np,
            "wsel": wsel_np,
            "biasx": bias_np,
        }
        for m in range(NCORES)
    ]
    import os

    trace = bool(int(os.environ.get("KERNEL_TRACE", "0")))
    res = run_bass_kernel_spmd(
        nc, in_maps, core_ids=list(range(NCORES)), trace=trace
    )
    LAST_RESULT[0] = res

    # unshard: outT [8=(t,h), 128=(s_loc,o_loc), VP] per core
    parts = []
    for m in range(NCORES):
        r = res.results[m]["outT"]  # [8, 128, VP]
        r = r.reshape(4, 2, 16, 8, VP)[:, :, :, :, :V]  # [t, h, sl, ol, v]
        # o = 8h + ol ; s_local_in_core = 16t + sl
        r = r.transpose(1, 3, 4, 0, 2).reshape(FOUT, V, SLOC)  # [o, v, s]
        parts.append(r)
    out = np.concatenate(parts, axis=2)  # [o, v, 512]
    return np.ascontiguousarray(
        out.reshape(1, FOUT, V, 8, 8, 8).astype(np.float32)
    )
